# revision 20
# baseline (speedup 1.0000x reference)
"""Trainium2 Bass kernel for nn_CrossWinAttention (windowed cross attention).

Contract: kernel(**inputs) takes FULL numpy inputs (as produced by
setup_inputs()) and returns the FULL output of the reference nn.Module.

Sharding: the (b, x*y) = 2*64 = 128 window cells are fully independent
(LN, QKV proj, per-cell attention, proj, mean over n are all cell-local).
16 cells per core on 8 NeuronCores; no collectives.

Per-cell schedule (tokens t = 64n + 8*w1 + w2; 3 chunks c of 128, n=2c+o):
  x      [128p, 3t, 3c, 128d] f16   host-packed token-major input
  LN     bn_stats/bn_aggr (DVE), rstd via bitcast-rsqrt + 1 Newton (DVE)
  xT     [128d, 9, 128p]            one blocked DMA-transpose per cell
  qT/kT  [H*DH, 384t] = W.T @ xT    (head h on partitions 32h:32h+32)
  vf     [128t, 3c, H*DH]           v-proj computed token-major directly
                                    (lhsT = xT chunk, rhs = Wv)
  sc     [128k, 3qt, 4h, 128q] PSUM per kc; exp -> E f16 SBUF (ACT)
  a/l    E chunk STATIONARY, rhs = vf slice (N=32) / ones (N=1), PSUM acc
  an     a * recip(l) (DVE reciprocal + broadcast multiply)
  amT    mean over n via indicator matmul (N=64), z = Wp.T @ amT (N=64)
  skip/bias are added on the host during the unshard (O(output)).
"""

import sys

sys.path.insert(0, "/opt/trn_rl_repo")

import contextlib

import numpy as np

import concourse.bass as bass
import concourse.tile as tile
from concourse import bacc, mybir
from concourse.bass_utils import run_bass_kernel_spmd

# Problem shape (hardcoded per spec nn_CrossWinAttention_81346680586407)
B, N, X, Y, W1, W2, D = 2, 6, 8, 8, 8, 8, 128
H, DH = 4, 32
EPS = 1e-5
L = X * Y              # 64 cells per batch
Q = N * W1 * W2        # 384 tokens per cell
W = W1 * W2            # 64 output positions per cell
NCORES = 8
CELLS = (B * L) // NCORES   # 16 cells per core
GRP = 4                     # cells per input-DMA batch
F32 = mybir.dt.float32
F16 = mybir.dt.float16
I32 = mybir.dt.int32
AF = mybir.ActivationFunctionType
ALU = mybir.AluOpType

RSQRT_MAGIC = 0x5F3759DF


def _build(n_cells: int):
    """Build the per-core Bass graph (same program on all 8 cores)."""
    nc = bacc.Bacc("TRN2", target_bir_lowering=False, debug=False,
                   num_devices=NCORES)

    x_in = nc.dram_tensor("x_in", [128, n_cells, 3, 3, D], F16,
                          kind="ExternalInput").ap()
    cst_d = nc.dram_tensor("cst", [128, 1168], mybir.dt.uint8,
                           kind="ExternalInput").ap()
    out_d = nc.dram_tensor("out", [128, n_cells, W], F32,
                           kind="ExternalOutput").ap()

    grp_sizes = [GRP] * ((n_cells + GRP - 1) // GRP)

    with tile.TileContext(nc) as tc, contextlib.ExitStack() as ctx:
        cpool = ctx.enter_context(tc.tile_pool(name="consts", bufs=1))
        xin = ctx.enter_context(tc.tile_pool(name="xin", bufs=3))
        stp = ctx.enter_context(tc.tile_pool(name="stats", bufs=3))
        xnp = ctx.enter_context(tc.tile_pool(name="xn", bufs=3))
        xtp = ctx.enter_context(tc.tile_pool(name="xt", bufs=3))
        qkvp = ctx.enter_context(tc.tile_pool(name="qkv", bufs=3))
        epool = ctx.enter_context(tc.tile_pool(name="E", bufs=9))
        anp = ctx.enter_context(tc.tile_pool(name="an", bufs=3))
        zop = ctx.enter_context(tc.tile_pool(name="zo", bufs=2))
        pp_ps = ctx.enter_context(tc.tile_pool(name="pp_ps", bufs=1, space="PSUM"))
        sc_ps = ctx.enter_context(tc.tile_pool(name="sc_ps", bufs=2, space="PSUM"))
        big_ps = ctx.enter_context(tc.tile_pool(name="big_ps", bufs=1, space="PSUM"))

        # --- constants: one byte-packed DMA, sliced via bitcast ------------
        cst = cpool.tile([128, 1168], mybir.dt.uint8)
        c16 = cst[:, 0:1154].bitcast(F16)
        wq_sb = c16[:, 0:128]
        wk_sb = c16[:, 128:256]
        wv_sb = c16[:, 256:384]
        wp_sb = c16[:, 384:512]
        sel_sb = c16[:, 512:576]
        ones_sb = c16[:, 576:577]
        bq_sb = cst[:, 1156:1160].bitcast(F32)
        bk_sb = cst[:, 1160:1164].bitcast(F32)
        ci32 = cst[:, 1164:1168].bitcast(I32)

        cell0 = 0
        for g, ncell_g in enumerate(grp_sizes):
          gs = slice(cell0, cell0 + ncell_g)
          xin4 = xin.tile([128, ncell_g, 3, 3, D], F16, tag="xin")
          nc.sync.dma_start(xin4[:], x_in[:, gs])
          if g == 0:
              nc.sync.dma_start(cst[:], cst_d[:])
          zo4 = zop.tile([D, ncell_g, W], F32, tag="zo")
          for j in range(ncell_g):
            i = cell0 + j
            x3 = xin4[:, j]

            # ---- LayerNorm stats (multi-group bn_stats per tensor)
            st = stp.tile([128, 3, 3, 6], F32, tag="st")
            mv = stp.tile([128, 3, 3, 2], F32, tag="mv")
            for t in range(3):
                nc.vector.bn_stats(st[:, t], x3[:, t])
                for c in range(3):
                    nc.vector.bn_aggr(mv[:, t, c], st[:, t, c])

            # ---- rstd = 1/sqrt(var+eps): bitcast rsqrt + 1 Newton (DVE)
            ve = stp.tile([128, 3, 3, 1], F32, tag="ve")
            nc.vector.tensor_scalar(ve[:], mv[:, :, :, 1:2], EPS, None,
                                    op0=ALU.add)
            ti = stp.tile([128, 3, 3, 1], I32, tag="ti")
            nc.vector.tensor_scalar(ti[:], ve[:].bitcast(I32), 1, None,
                                    op0=ALU.logical_shift_right)
            y0i = stp.tile([128, 3, 3, 1], I32, tag="y0i")
            magic = ci32[:, 0:1].unsqueeze(2).unsqueeze(3)
            nc.vector.tensor_tensor(y0i[:], magic.broadcast_to([128, 3, 3, 1]),
                                    ti[:], op=ALU.subtract)
            y0 = y0i[:].bitcast(F32)
            t2 = stp.tile([128, 3, 3, 1], F32, tag="t2")
            t3 = stp.tile([128, 3, 3, 1], F32, tag="t3")
            rstd = stp.tile([128, 3, 3, 1], F32, tag="rstd")
            nc.vector.tensor_mul(t2[:], y0, y0)
            nc.vector.tensor_mul(t3[:], ve[:], t2[:])
            nc.vector.tensor_scalar(t2[:], t3[:], -0.5, 1.5,
                                    op0=ALU.mult, op1=ALU.add)
            nc.vector.tensor_mul(rstd[:], y0, t2[:])

            # ---- normalize (x-m)*rstd -> f16 (DVE, 4x mode)
            xn = xnp.tile([128, 3, 3, D], F16, tag="xn")
            for t in range(3):
                for c in range(3):
                    nc.vector.tensor_scalar(
                        xn[:, t, c], x3[:, t, c],
                        mv[:, t, c, 0:1], rstd[:, t, c],
                        op0=ALU.subtract, op1=ALU.mult)

            # ---- one blocked DMA-transpose for q,k,v
            xT = xtp.tile([D, 9, 128], F16, tag="xT")
            nc.sync.dma_start_transpose(
                xT[:], xn[:].rearrange("p a b d -> p (a b d)"))

            # ---- q/k projections: [H*DH, 384] = W.T @ xT
            qT_sb = qkvp.tile([H * DH, Q], F16, tag="qT")
            kT_sb = qkvp.tile([H * DH, Q], F16, tag="kT")
            for w_ap, b_ap, dst, t in ((wq_sb, bq_sb, qT_sb, 0),
                                       (wk_sb, bk_sb, kT_sb, 1)):
                pp = pp_ps.tile([128, Q], F32, tag="pp")
                nc.tensor.matmul(pp[:], w_ap,
                                 xT[:, 3 * t:3 * t + 3, :].rearrange(
                                     "d a p -> d (a p)"),
                                 start=True, stop=True)
                nc.gpsimd.tensor_scalar(dst[:], pp[:], b_ap, None,
                                        op0=ALU.add)
            # v projection directly token-major: lhsT = xT chunk
            ppv = pp_ps.tile([128, 3, H * DH], F32, tag="pp")
            for c in range(3):
                nc.tensor.matmul(ppv[:, c, :], xT[:, 6 + c, :], wv_sb,
                                 start=True, stop=True)
            vf_sb = qkvp.tile([128, 3, H * DH], F16, tag="vf")
            nc.gpsimd.tensor_copy(vf_sb[:], ppv[:])

            # ---- scores + exp per k-chunk
            E_ts = []
            for kc in range(3):
                sc = sc_ps.tile([128, 3, 4, 128], F32, tag="sc")
                for qt in range(3):
                    for h in range(4):
                        nc.tensor.matmul(
                            sc[:, qt, h, :],
                            kT_sb[32 * h:32 * h + 32,
                                  128 * kc:128 * kc + 128],
                            qT_sb[32 * h:32 * h + 32,
                                  128 * qt:128 * qt + 128],
                            start=True, stop=True,
                            tile_position=(32 * h, 0))
                E_t = epool.tile([128, 3, 4, 128], F16, tag="E")
                nc.scalar.activation(E_t[:], sc[:], AF.Exp)
                E_ts.append(E_t)

            # ---- A@V with E stationary; l rides along as N=1 matmuls
            # amT/z reuse the a-chunk-0 region after an-norm has read it
            big = big_ps.tile([128, 396], F32, tag="big")
            a_ap = big[:, 0:384].rearrange("p (qt h e) -> p qt h e",
                                           h=4, e=32)
            l_ap = big[:, 384:396].rearrange("p (qt h) -> p qt h", h=4)
            amT_ap = big[:, 0:W]
            z_ap = big[:, W:2 * W]
            for kc in range(3):
                st_f = (kc == 0)
                sp_f = (kc == 2)
                for qt in range(3):
                    for h in range(4):
                        Ech = E_ts[kc][:, qt, h, :]
                        nc.tensor.matmul(a_ap[:, qt, h, :], Ech,
                                         vf_sb[:, kc, 32 * h:32 * h + 32],
                                         start=st_f, stop=sp_f)
                        nc.tensor.matmul(l_ap[:, qt, h:h + 1], Ech,
                                         ones_sb,
                                         start=st_f, stop=sp_f)

            # ---- normalize: an = a * recip(l) (recip on DVE, mult on Pool)
            rcp = anp.tile([128, 3, 4], F32, tag="rcp")
            nc.vector.reciprocal(rcp[:], l_ap)
            an_sb = anp.tile([128, 3, 4, 32], F16, tag="an")
            for qt in range(3):
                nc.vector.tensor_mul(
                    an_sb[:, qt], a_ap[:, qt],
                    rcp[:, qt].unsqueeze(2).broadcast_to([128, 4, 32]))

            # ---- mean over n (indicator matmul) + output projection
            for qt in range(3):
                nc.tensor.matmul(amT_ap, an_sb[:, qt].rearrange(
                    "p h e -> p (h e)"), sel_sb,
                    start=(qt == 0), stop=(qt == 2))
            amT_sb = anp.tile([128, W], F16, tag="amT")
            nc.gpsimd.tensor_copy(amT_sb[:], amT_ap)
            nc.tensor.matmul(z_ap, wp_sb, amT_sb[:], start=True, stop=True)
            nc.gpsimd.tensor_copy(zo4[:, j, :], z_ap)
          nc.gpsimd.dma_start(out_d[:, gs], zo4[:])
          cell0 += ncell_g

    nc.compile()
    return nc


_NC_CACHE = {}


def _get_nc(n_cells: int):
    if n_cells not in _NC_CACHE:
        _NC_CACHE[n_cells] = _build(n_cells)
    return _NC_CACHE[n_cells]


def _fold_weights(head_gate, lnq_g, lnq_b, lnk_g, lnk_b, lnv_g, lnv_b,
                  Wq, bq, Wk, bk, Wv, bv, Wp, bp):
    """Fold LN affine, head gate, scale, and 1/6-mean into the weights."""
    scale = DH ** -0.5
    gh = np.repeat(np.asarray(head_gate, np.float64), DH)        # [H*DH]
    sq = np.sqrt(scale)

    def proj(g, b, Wx, bx, colscale):
        Wf = (np.asarray(g, np.float64)[:, None] * np.asarray(Wx, np.float64)) * colscale
        bf = (np.asarray(b, np.float64) @ np.asarray(Wx, np.float64)
              + np.asarray(bx, np.float64)) * colscale
        return Wf, bf

    Wq2, bq2 = proj(lnq_g, lnq_b, Wq, bq, gh * sq)
    Wk2, bk2 = proj(lnk_g, lnk_b, Wk, bk, gh * sq)
    Wv2, bv2 = proj(lnv_g, lnv_b, Wv, bv, gh)
    Wp2 = np.asarray(Wp, np.float64) / 6.0
    bp2 = np.asarray(bp, np.float64) + bv2 @ np.asarray(Wp, np.float64)
    return (Wq2.astype(np.float16), bq2.astype(np.float32),
            Wk2.astype(np.float16), bk2.astype(np.float32),
            Wv2.astype(np.float16),
            Wp2.astype(np.float16), bp2.astype(np.float32))


def make_in_maps(q, k, v, skip, head_gate,
                 lnq_g, lnq_b, lnk_g, lnk_b, lnv_g, lnv_b,
                 Wq, bq, Wk, bk, Wv, bv, Wp, bp):
    q = np.asarray(q); k = np.asarray(k); v = np.asarray(v)
    Wq2, bq2, Wk2, bk2, Wv2, Wp2, bp2 = _fold_weights(
        head_gate, lnq_g, lnq_b, lnk_g, lnk_b, lnv_g, lnv_b,
        Wq, bq, Wk, bk, Wv, bv, Wp, bp)

    # device layout: x_in[p=(o,w1,w2), cell=(b,x,y), tensor, c, d], n = 2c+o
    arr = np.stack([q, k, v], axis=1)               # [B, 3, N, X, Y, W1, W2, D]
    arr = arr.reshape(B, 3, 3, 2, X, Y, W1, W2, D)  # N -> (c, o)
    arr = arr.transpose(3, 6, 7, 0, 4, 5, 1, 2, 8)  # [o, w1, w2, B, X, Y, 3, c, d]
    arr = np.ascontiguousarray(
        arr.reshape(128, B * L, 3, 3, D).astype(np.float16))

    sel = np.zeros((128, W), np.float16)
    sel[np.arange(128), np.arange(128) % W] = 1.0
    c16 = np.concatenate(
        [Wq2, Wk2, Wv2, Wp2, sel, np.ones((128, 1), np.float16)],
        axis=1)                                      # [128, 577]
    cst = np.zeros((128, 1168), np.uint8)
    cst[:, 0:1154] = c16.view(np.uint8)
    cst[:, 1156:1160] = bq2.reshape(128, 1).view(np.uint8)
    cst[:, 1160:1164] = bk2.reshape(128, 1).view(np.uint8)
    cst[:, 1164:1168] = np.full((128, 1), RSQRT_MAGIC, np.int32).view(np.uint8)

    consts = {"cst": cst}
    in_maps = []
    for r in range(NCORES):
        s = slice(r * CELLS, (r + 1) * CELLS)
        in_maps.append({
            "x_in": np.ascontiguousarray(arr[:, s]),
            **consts,
        })
    return in_maps, bp2


def kernel(**inputs):
    in_maps, bp2 = make_in_maps(**inputs)
    skip = np.asarray(inputs["skip"])
    nc = _get_nc(CELLS)
    res = run_bass_kernel_spmd(nc, in_maps, core_ids=list(range(NCORES)))
    outs = np.stack([res.results[r]["out"] for r in range(NCORES)])  # [8,D,16,W]
    z = outs.transpose(0, 2, 3, 1).reshape(B * L, W, D)              # [128,W,D]
    z = z + bp2[None, None, :].astype(np.float32)
    z = z.reshape(B, X, Y, W1, W2, D).astype(np.float32) + skip
    return z


# revision 21
# speedup vs baseline: 1.0024x; 1.0024x over previous
"""Trainium2 Bass kernel for nn_CrossWinAttention (windowed cross attention).

Contract: kernel(**inputs) takes FULL numpy inputs (as produced by
setup_inputs()) and returns the FULL output of the reference nn.Module.

Sharding: the (b, x*y) = 2*64 = 128 window cells are fully independent
(LN, QKV proj, per-cell attention, proj, mean over n are all cell-local).
16 cells per core on 8 NeuronCores; no collectives.

Per-cell schedule (tokens t = 64n + 8*w1 + w2; 3 chunks c of 128, n=2c+o):
  x      [128p, 3t, 3c, 128d] f16   host-packed token-major input
  LN     bn_stats/bn_aggr (DVE), rstd via bitcast-rsqrt + 1 Newton (DVE)
  xT     [128d, 9, 128p]            one blocked DMA-transpose per cell
  qT/kT  [H*DH, 384t] = W.T @ xT    (head h on partitions 32h:32h+32)
  vf     [128t, 3c, H*DH]           v-proj computed token-major directly
                                    (lhsT = xT chunk, rhs = Wv)
  sc     [128k, 3qt, 4h, 128q] PSUM per kc; exp -> E f16 SBUF (ACT)
  a/l    E chunk STATIONARY, rhs = vf slice (N=32) / ones (N=1), PSUM acc
  an     a * recip(l) (DVE reciprocal + broadcast multiply)
  amT    mean over n via indicator matmul (N=64), z = Wp.T @ amT (N=64)
  skip/bias are added on the host during the unshard (O(output)).
"""

import sys

sys.path.insert(0, "/opt/trn_rl_repo")

import contextlib

import numpy as np

import concourse.bass as bass
import concourse.tile as tile
from concourse import bacc, mybir
from concourse.bass_utils import run_bass_kernel_spmd

# Problem shape (hardcoded per spec nn_CrossWinAttention_81346680586407)
B, N, X, Y, W1, W2, D = 2, 6, 8, 8, 8, 8, 128
H, DH = 4, 32
EPS = 1e-5
L = X * Y              # 64 cells per batch
Q = N * W1 * W2        # 384 tokens per cell
W = W1 * W2            # 64 output positions per cell
NCORES = 8
CELLS = (B * L) // NCORES   # 16 cells per core
GRP = 4                     # cells per input-DMA batch
F32 = mybir.dt.float32
F16 = mybir.dt.float16
I32 = mybir.dt.int32
AF = mybir.ActivationFunctionType
ALU = mybir.AluOpType

RSQRT_MAGIC = 0x5F3759DF


def _build(n_cells: int):
    """Build the per-core Bass graph (same program on all 8 cores)."""
    nc = bacc.Bacc("TRN2", target_bir_lowering=False, debug=False,
                   num_devices=NCORES)

    x_in = nc.dram_tensor("x_in", [128, n_cells, 3, 3, D], F16,
                          kind="ExternalInput").ap()
    cst_d = nc.dram_tensor("cst", [128, 1168], mybir.dt.uint8,
                           kind="ExternalInput").ap()
    out_d = nc.dram_tensor("out", [128, n_cells, W], F32,
                           kind="ExternalOutput").ap()

    grp_sizes = [1, 3, 4, 4, 4] if n_cells == 16 else [GRP] * (
        (n_cells + GRP - 1) // GRP)

    with tile.TileContext(nc) as tc, contextlib.ExitStack() as ctx:
        cpool = ctx.enter_context(tc.tile_pool(name="consts", bufs=1))
        xin = ctx.enter_context(tc.tile_pool(name="xin", bufs=3))
        stp = ctx.enter_context(tc.tile_pool(name="stats", bufs=3))
        xnp = ctx.enter_context(tc.tile_pool(name="xn", bufs=3))
        xtp = ctx.enter_context(tc.tile_pool(name="xt", bufs=3))
        qkvp = ctx.enter_context(tc.tile_pool(name="qkv", bufs=3))
        epool = ctx.enter_context(tc.tile_pool(name="E", bufs=9))
        anp = ctx.enter_context(tc.tile_pool(name="an", bufs=3))
        zop = ctx.enter_context(tc.tile_pool(name="zo", bufs=2))
        pp_ps = ctx.enter_context(tc.tile_pool(name="pp_ps", bufs=1, space="PSUM"))
        sc_ps = ctx.enter_context(tc.tile_pool(name="sc_ps", bufs=2, space="PSUM"))
        big_ps = ctx.enter_context(tc.tile_pool(name="big_ps", bufs=1, space="PSUM"))

        # --- constants: one byte-packed DMA, sliced via bitcast ------------
        cst = cpool.tile([128, 1168], mybir.dt.uint8)
        c16 = cst[:, 0:1154].bitcast(F16)
        wq_sb = c16[:, 0:128]
        wk_sb = c16[:, 128:256]
        wv_sb = c16[:, 256:384]
        wp_sb = c16[:, 384:512]
        sel_sb = c16[:, 512:576]
        ones_sb = c16[:, 576:577]
        bq_sb = cst[:, 1156:1160].bitcast(F32)
        bk_sb = cst[:, 1160:1164].bitcast(F32)
        ci32 = cst[:, 1164:1168].bitcast(I32)

        cell0 = 0
        for g, ncell_g in enumerate(grp_sizes):
          gs = slice(cell0, cell0 + ncell_g)
          xin4 = xin.tile([128, ncell_g, 3, 3, D], F16, tag="xin")
          nc.sync.dma_start(xin4[:], x_in[:, gs])
          if g == 0:
              nc.sync.dma_start(cst[:], cst_d[:])
          zo4 = zop.tile([D, ncell_g, W], F32, tag="zo")
          for j in range(ncell_g):
            i = cell0 + j
            x3 = xin4[:, j]

            # ---- LayerNorm stats (multi-group bn_stats per tensor)
            st = stp.tile([128, 3, 3, 6], F32, tag="st")
            mv = stp.tile([128, 3, 3, 2], F32, tag="mv")
            for t in range(3):
                nc.vector.bn_stats(st[:, t], x3[:, t])
                for c in range(3):
                    nc.vector.bn_aggr(mv[:, t, c], st[:, t, c])

            # ---- rstd = 1/sqrt(var+eps): bitcast rsqrt + 1 Newton (DVE)
            ve = stp.tile([128, 3, 3, 1], F32, tag="ve")
            nc.vector.tensor_scalar(ve[:], mv[:, :, :, 1:2], EPS, None,
                                    op0=ALU.add)
            ti = stp.tile([128, 3, 3, 1], I32, tag="ti")
            nc.vector.tensor_scalar(ti[:], ve[:].bitcast(I32), 1, None,
                                    op0=ALU.logical_shift_right)
            y0i = stp.tile([128, 3, 3, 1], I32, tag="y0i")
            magic = ci32[:, 0:1].unsqueeze(2).unsqueeze(3)
            nc.vector.tensor_tensor(y0i[:], magic.broadcast_to([128, 3, 3, 1]),
                                    ti[:], op=ALU.subtract)
            y0 = y0i[:].bitcast(F32)
            t2 = stp.tile([128, 3, 3, 1], F32, tag="t2")
            t3 = stp.tile([128, 3, 3, 1], F32, tag="t3")
            rstd = stp.tile([128, 3, 3, 1], F32, tag="rstd")
            nc.vector.tensor_mul(t2[:], y0, y0)
            nc.vector.tensor_mul(t3[:], ve[:], t2[:])
            nc.vector.tensor_scalar(t2[:], t3[:], -0.5, 1.5,
                                    op0=ALU.mult, op1=ALU.add)
            nc.vector.tensor_mul(rstd[:], y0, t2[:])

            # ---- normalize (x-m)*rstd -> f16 (DVE, 4x mode)
            xn = xnp.tile([128, 3, 3, D], F16, tag="xn")
            for t in range(3):
                for c in range(3):
                    nc.vector.tensor_scalar(
                        xn[:, t, c], x3[:, t, c],
                        mv[:, t, c, 0:1], rstd[:, t, c],
                        op0=ALU.subtract, op1=ALU.mult)

            # ---- one blocked DMA-transpose for q,k,v
            xT = xtp.tile([D, 9, 128], F16, tag="xT")
            nc.sync.dma_start_transpose(
                xT[:], xn[:].rearrange("p a b d -> p (a b d)"))

            # ---- q/k projections: [H*DH, 384] = W.T @ xT
            qT_sb = qkvp.tile([H * DH, Q], F16, tag="qT")
            kT_sb = qkvp.tile([H * DH, Q], F16, tag="kT")
            for w_ap, b_ap, dst, t in ((wq_sb, bq_sb, qT_sb, 0),
                                       (wk_sb, bk_sb, kT_sb, 1)):
                pp = pp_ps.tile([128, Q], F32, tag="pp")
                nc.tensor.matmul(pp[:], w_ap,
                                 xT[:, 3 * t:3 * t + 3, :].rearrange(
                                     "d a p -> d (a p)"),
                                 start=True, stop=True)
                nc.gpsimd.tensor_scalar(dst[:], pp[:], b_ap, None,
                                        op0=ALU.add)
            # v projection directly token-major: lhsT = xT chunk
            ppv = pp_ps.tile([128, 3, H * DH], F32, tag="pp")
            for c in range(3):
                nc.tensor.matmul(ppv[:, c, :], xT[:, 6 + c, :], wv_sb,
                                 start=True, stop=True)
            vf_sb = qkvp.tile([128, 3, H * DH], F16, tag="vf")
            nc.gpsimd.tensor_copy(vf_sb[:], ppv[:])

            # ---- scores + exp per k-chunk
            E_ts = []
            for kc in range(3):
                sc = sc_ps.tile([128, 3, 4, 128], F32, tag="sc")
                for qt in range(3):
                    for h in range(4):
                        nc.tensor.matmul(
                            sc[:, qt, h, :],
                            kT_sb[32 * h:32 * h + 32,
                                  128 * kc:128 * kc + 128],
                            qT_sb[32 * h:32 * h + 32,
                                  128 * qt:128 * qt + 128],
                            start=True, stop=True,
                            tile_position=(32 * h, 0))
                E_t = epool.tile([128, 3, 4, 128], F16, tag="E")
                nc.scalar.activation(E_t[:], sc[:], AF.Exp)
                E_ts.append(E_t)

            # ---- A@V with E stationary; l rides along as N=1 matmuls
            # amT/z reuse the a-chunk-0 region after an-norm has read it
            big = big_ps.tile([128, 396], F32, tag="big")
            a_ap = big[:, 0:384].rearrange("p (qt h e) -> p qt h e",
                                           h=4, e=32)
            l_ap = big[:, 384:396].rearrange("p (qt h) -> p qt h", h=4)
            amT_ap = big[:, 0:W]
            z_ap = big[:, W:2 * W]
            for kc in range(3):
                st_f = (kc == 0)
                sp_f = (kc == 2)
                for qt in range(3):
                    for h in range(4):
                        Ech = E_ts[kc][:, qt, h, :]
                        nc.tensor.matmul(a_ap[:, qt, h, :], Ech,
                                         vf_sb[:, kc, 32 * h:32 * h + 32],
                                         start=st_f, stop=sp_f)
                        nc.tensor.matmul(l_ap[:, qt, h:h + 1], Ech,
                                         ones_sb,
                                         start=st_f, stop=sp_f)

            # ---- normalize: an = a * recip(l) (recip on DVE, mult on Pool)
            rcp = anp.tile([128, 3, 4], F32, tag="rcp")
            nc.vector.reciprocal(rcp[:], l_ap)
            an_sb = anp.tile([128, 3, 4, 32], F16, tag="an")
            for qt in range(3):
                nc.vector.tensor_mul(
                    an_sb[:, qt], a_ap[:, qt],
                    rcp[:, qt].unsqueeze(2).broadcast_to([128, 4, 32]))

            # ---- mean over n (indicator matmul) + output projection
            for qt in range(3):
                nc.tensor.matmul(amT_ap, an_sb[:, qt].rearrange(
                    "p h e -> p (h e)"), sel_sb,
                    start=(qt == 0), stop=(qt == 2))
            amT_sb = anp.tile([128, W], F16, tag="amT")
            nc.gpsimd.tensor_copy(amT_sb[:], amT_ap)
            nc.tensor.matmul(z_ap, wp_sb, amT_sb[:], start=True, stop=True)
            nc.gpsimd.tensor_copy(zo4[:, j, :], z_ap)
          nc.sync.dma_start(out_d[:, gs], zo4[:])
          cell0 += ncell_g

    nc.compile()
    return nc


_NC_CACHE = {}


def _get_nc(n_cells: int):
    if n_cells not in _NC_CACHE:
        _NC_CACHE[n_cells] = _build(n_cells)
    return _NC_CACHE[n_cells]


def _fold_weights(head_gate, lnq_g, lnq_b, lnk_g, lnk_b, lnv_g, lnv_b,
                  Wq, bq, Wk, bk, Wv, bv, Wp, bp):
    """Fold LN affine, head gate, scale, and 1/6-mean into the weights."""
    scale = DH ** -0.5
    gh = np.repeat(np.asarray(head_gate, np.float64), DH)        # [H*DH]
    sq = np.sqrt(scale)

    def proj(g, b, Wx, bx, colscale):
        Wf = (np.asarray(g, np.float64)[:, None] * np.asarray(Wx, np.float64)) * colscale
        bf = (np.asarray(b, np.float64) @ np.asarray(Wx, np.float64)
              + np.asarray(bx, np.float64)) * colscale
        return Wf, bf

    Wq2, bq2 = proj(lnq_g, lnq_b, Wq, bq, gh * sq)
    Wk2, bk2 = proj(lnk_g, lnk_b, Wk, bk, gh * sq)
    Wv2, bv2 = proj(lnv_g, lnv_b, Wv, bv, gh)
    Wp2 = np.asarray(Wp, np.float64) / 6.0
    bp2 = np.asarray(bp, np.float64) + bv2 @ np.asarray(Wp, np.float64)
    return (Wq2.astype(np.float16), bq2.astype(np.float32),
            Wk2.astype(np.float16), bk2.astype(np.float32),
            Wv2.astype(np.float16),
            Wp2.astype(np.float16), bp2.astype(np.float32))


def make_in_maps(q, k, v, skip, head_gate,
                 lnq_g, lnq_b, lnk_g, lnk_b, lnv_g, lnv_b,
                 Wq, bq, Wk, bk, Wv, bv, Wp, bp):
    q = np.asarray(q); k = np.asarray(k); v = np.asarray(v)
    Wq2, bq2, Wk2, bk2, Wv2, Wp2, bp2 = _fold_weights(
        head_gate, lnq_g, lnq_b, lnk_g, lnk_b, lnv_g, lnv_b,
        Wq, bq, Wk, bk, Wv, bv, Wp, bp)

    # device layout: x_in[p=(o,w1,w2), cell=(b,x,y), tensor, c, d], n = 2c+o
    arr = np.stack([q, k, v], axis=1)               # [B, 3, N, X, Y, W1, W2, D]
    arr = arr.reshape(B, 3, 3, 2, X, Y, W1, W2, D)  # N -> (c, o)
    arr = arr.transpose(3, 6, 7, 0, 4, 5, 1, 2, 8)  # [o, w1, w2, B, X, Y, 3, c, d]
    arr = np.ascontiguousarray(
        arr.reshape(128, B * L, 3, 3, D).astype(np.float16))

    sel = np.zeros((128, W), np.float16)
    sel[np.arange(128), np.arange(128) % W] = 1.0
    c16 = np.concatenate(
        [Wq2, Wk2, Wv2, Wp2, sel, np.ones((128, 1), np.float16)],
        axis=1)                                      # [128, 577]
    cst = np.zeros((128, 1168), np.uint8)
    cst[:, 0:1154] = c16.view(np.uint8)
    cst[:, 1156:1160] = bq2.reshape(128, 1).view(np.uint8)
    cst[:, 1160:1164] = bk2.reshape(128, 1).view(np.uint8)
    cst[:, 1164:1168] = np.full((128, 1), RSQRT_MAGIC, np.int32).view(np.uint8)

    consts = {"cst": cst}
    in_maps = []
    for r in range(NCORES):
        s = slice(r * CELLS, (r + 1) * CELLS)
        in_maps.append({
            "x_in": np.ascontiguousarray(arr[:, s]),
            **consts,
        })
    return in_maps, bp2


def kernel(**inputs):
    in_maps, bp2 = make_in_maps(**inputs)
    skip = np.asarray(inputs["skip"])
    nc = _get_nc(CELLS)
    res = run_bass_kernel_spmd(nc, in_maps, core_ids=list(range(NCORES)))
    outs = np.stack([res.results[r]["out"] for r in range(NCORES)])  # [8,D,16,W]
    z = outs.transpose(0, 2, 3, 1).reshape(B * L, W, D)              # [128,W,D]
    z = z + bp2[None, None, :].astype(np.float32)
    z = z.reshape(B, X, Y, W1, W2, D).astype(np.float32) + skip
    return z


# revision 22
# speedup vs baseline: 1.0219x; 1.0194x over previous
"""Trainium2 Bass kernel for nn_CrossWinAttention (windowed cross attention).

Contract: kernel(**inputs) takes FULL numpy inputs (as produced by
setup_inputs()) and returns the FULL output of the reference nn.Module.

Sharding: the (b, x*y) = 2*64 = 128 window cells are fully independent
(LN, QKV proj, per-cell attention, proj, mean over n are all cell-local).
16 cells per core on 8 NeuronCores; no collectives.

Per-cell schedule (tokens t = 64n + 8*w1 + w2; 3 chunks c of 128, n=2c+o):
  x      [128p, 3t, 3c, 128d] f16   host-packed token-major input
  LN     bn_stats/bn_aggr (DVE), rstd via bitcast-rsqrt + 1 Newton (DVE)
  xT     [128d, 9, 128p]            one blocked DMA-transpose per cell
  qT/kT  [H*DH, 384t] = W.T @ xT    (head h on partitions 32h:32h+32)
  vf     [128t, 3c, H*DH]           v-proj computed token-major directly
                                    (lhsT = xT chunk, rhs = Wv)
  sc     [128k, 3qt, 4h, 128q] PSUM per kc; exp -> E f16 SBUF (ACT)
  a/l    E chunk STATIONARY, rhs = vf slice (N=32) / ones (N=1), PSUM acc
  an     a * recip(l) (DVE reciprocal + broadcast multiply)
  amT    mean over n via indicator matmul (N=64), z = Wp.T @ amT (N=64)
  skip/bias are added on the host during the unshard (O(output)).
"""

import sys

sys.path.insert(0, "/opt/trn_rl_repo")

import contextlib

import numpy as np

import concourse.bass as bass
import concourse.tile as tile
from concourse import bacc, mybir
from concourse.bass_utils import run_bass_kernel_spmd

# Problem shape (hardcoded per spec nn_CrossWinAttention_81346680586407)
B, N, X, Y, W1, W2, D = 2, 6, 8, 8, 8, 8, 128
H, DH = 4, 32
EPS = 1e-5
L = X * Y              # 64 cells per batch
Q = N * W1 * W2        # 384 tokens per cell
W = W1 * W2            # 64 output positions per cell
NCORES = 8
CELLS = (B * L) // NCORES   # 16 cells per core
GRP = 4                     # cells per input-DMA batch
F32 = mybir.dt.float32
F16 = mybir.dt.float16
I32 = mybir.dt.int32
AF = mybir.ActivationFunctionType
ALU = mybir.AluOpType

RSQRT_MAGIC = 0x5F3759DF


def _build(n_cells: int):
    """Build the per-core Bass graph (same program on all 8 cores)."""
    nc = bacc.Bacc("TRN2", target_bir_lowering=False, debug=False,
                   num_devices=NCORES)

    x_in = nc.dram_tensor("x_in", [128, n_cells, 3, 3, D], F16,
                          kind="ExternalInput").ap()
    cst_d = nc.dram_tensor("cst", [128, 1168], mybir.dt.uint8,
                           kind="ExternalInput").ap()
    out_d = nc.dram_tensor("out", [128, n_cells, W], F32,
                           kind="ExternalOutput").ap()

    grp_sizes = [GRP] * ((n_cells + GRP - 1) // GRP)

    with tile.TileContext(nc) as tc, contextlib.ExitStack() as ctx:
        cpool = ctx.enter_context(tc.tile_pool(name="consts", bufs=1))
        xin = ctx.enter_context(tc.tile_pool(name="xin", bufs=3))
        stp = ctx.enter_context(tc.tile_pool(name="stats", bufs=4))
        xnp = ctx.enter_context(tc.tile_pool(name="xn", bufs=4))
        xtp = ctx.enter_context(tc.tile_pool(name="xt", bufs=4))
        qkvp = ctx.enter_context(tc.tile_pool(name="qkv", bufs=4))
        epool = ctx.enter_context(tc.tile_pool(name="E", bufs=9))
        anp = ctx.enter_context(tc.tile_pool(name="an", bufs=3))
        zop = ctx.enter_context(tc.tile_pool(name="zo", bufs=2))
        pp_ps = ctx.enter_context(tc.tile_pool(name="pp_ps", bufs=1, space="PSUM"))
        sc_ps = ctx.enter_context(tc.tile_pool(name="sc_ps", bufs=2, space="PSUM"))
        big_ps = ctx.enter_context(tc.tile_pool(name="big_ps", bufs=1, space="PSUM"))

        # --- constants: one byte-packed DMA, sliced via bitcast ------------
        cst = cpool.tile([128, 1168], mybir.dt.uint8)
        c16 = cst[:, 0:1154].bitcast(F16)
        wq_sb = c16[:, 0:128]
        wk_sb = c16[:, 128:256]
        wv_sb = c16[:, 256:384]
        wp_sb = c16[:, 384:512]
        sel_sb = c16[:, 512:576]
        ones_sb = c16[:, 576:577]
        bq_sb = cst[:, 1156:1160].bitcast(F32)
        bk_sb = cst[:, 1160:1164].bitcast(F32)
        ci32 = cst[:, 1164:1168].bitcast(I32)

        cell0 = 0
        for g, ncell_g in enumerate(grp_sizes):
          gs = slice(cell0, cell0 + ncell_g)
          xin4 = xin.tile([128, ncell_g, 3, 3, D], F16, tag="xin")
          nc.sync.dma_start(xin4[:], x_in[:, gs])
          if g == 0:
              nc.sync.dma_start(cst[:], cst_d[:])
          zo4 = zop.tile([D, ncell_g, W], F32, tag="zo")
          for j in range(ncell_g):
            i = cell0 + j
            x3 = xin4[:, j]

            # ---- LayerNorm stats (multi-group bn_stats per tensor)
            st = stp.tile([128, 3, 3, 6], F32, tag="st")
            mv = stp.tile([128, 3, 3, 2], F32, tag="mv")
            for t in range(3):
                nc.vector.bn_stats(st[:, t], x3[:, t])
                for c in range(3):
                    nc.vector.bn_aggr(mv[:, t, c], st[:, t, c])

            # ---- rstd = 1/sqrt(var+eps): bitcast rsqrt + 1 Newton (DVE)
            ve = stp.tile([128, 3, 3, 1], F32, tag="ve")
            nc.vector.tensor_scalar(ve[:], mv[:, :, :, 1:2], EPS, None,
                                    op0=ALU.add)
            ti = stp.tile([128, 3, 3, 1], I32, tag="ti")
            nc.vector.tensor_scalar(ti[:], ve[:].bitcast(I32), 1, None,
                                    op0=ALU.logical_shift_right)
            y0i = stp.tile([128, 3, 3, 1], I32, tag="y0i")
            magic = ci32[:, 0:1].unsqueeze(2).unsqueeze(3)
            nc.vector.tensor_tensor(y0i[:], magic.broadcast_to([128, 3, 3, 1]),
                                    ti[:], op=ALU.subtract)
            y0 = y0i[:].bitcast(F32)
            t2 = stp.tile([128, 3, 3, 1], F32, tag="t2")
            t3 = stp.tile([128, 3, 3, 1], F32, tag="t3")
            rstd = stp.tile([128, 3, 3, 1], F32, tag="rstd")
            nc.vector.tensor_mul(t2[:], y0, y0)
            nc.vector.tensor_mul(t3[:], ve[:], t2[:])
            nc.vector.tensor_scalar(t2[:], t3[:], -0.5, 1.5,
                                    op0=ALU.mult, op1=ALU.add)
            nc.vector.tensor_mul(rstd[:], y0, t2[:])

            # ---- normalize (x-m)*rstd -> f16 (DVE, 4x mode)
            xn = xnp.tile([128, 3, 3, D], F16, tag="xn")
            for t in range(3):
                for c in range(3):
                    nc.vector.tensor_scalar(
                        xn[:, t, c], x3[:, t, c],
                        mv[:, t, c, 0:1], rstd[:, t, c],
                        op0=ALU.subtract, op1=ALU.mult)

            # ---- one blocked DMA-transpose for q,k,v
            xT = xtp.tile([D, 9, 128], F16, tag="xT")
            nc.sync.dma_start_transpose(
                xT[:], xn[:].rearrange("p a b d -> p (a b d)"))

            # ---- q/k projections: [H*DH, 384] = W.T @ xT
            qT_sb = qkvp.tile([H * DH, Q], F16, tag="qT")
            kT_sb = qkvp.tile([H * DH, Q], F16, tag="kT")
            for w_ap, b_ap, dst, t in ((wq_sb, bq_sb, qT_sb, 0),
                                       (wk_sb, bk_sb, kT_sb, 1)):
                pp = pp_ps.tile([128, Q], F32, tag="pp")
                nc.tensor.matmul(pp[:], w_ap,
                                 xT[:, 3 * t:3 * t + 3, :].rearrange(
                                     "d a p -> d (a p)"),
                                 start=True, stop=True)
                nc.gpsimd.tensor_scalar(dst[:], pp[:], b_ap, None,
                                        op0=ALU.add)
            # v projection directly token-major: lhsT = xT chunk
            ppv = pp_ps.tile([128, 3, H * DH], F32, tag="pp")
            for c in range(3):
                nc.tensor.matmul(ppv[:, c, :], xT[:, 6 + c, :], wv_sb,
                                 start=True, stop=True)
            vf_sb = qkvp.tile([128, 3, H * DH], F16, tag="vf")
            nc.gpsimd.tensor_copy(vf_sb[:], ppv[:])

            # ---- scores + exp per k-chunk
            E_ts = []
            for kc in range(3):
                sc = sc_ps.tile([128, 3, 4, 128], F32, tag="sc")
                for qt in range(3):
                    for h in range(4):
                        nc.tensor.matmul(
                            sc[:, qt, h, :],
                            kT_sb[32 * h:32 * h + 32,
                                  128 * kc:128 * kc + 128],
                            qT_sb[32 * h:32 * h + 32,
                                  128 * qt:128 * qt + 128],
                            start=True, stop=True,
                            tile_position=(32 * h, 0))
                E_t = epool.tile([128, 3, 4, 128], F16, tag="E")
                nc.scalar.activation(E_t[:], sc[:], AF.Exp)
                E_ts.append(E_t)

            # ---- A@V with E stationary; l rides along as N=1 matmuls
            # amT/z reuse the a-chunk-0 region after an-norm has read it
            big = big_ps.tile([128, 396], F32, tag="big")
            a_ap = big[:, 0:384].rearrange("p (qt h e) -> p qt h e",
                                           h=4, e=32)
            l_ap = big[:, 384:396].rearrange("p (qt h) -> p qt h", h=4)
            amT_ap = big[:, 0:W]
            z_ap = big[:, W:2 * W]
            for kc in range(3):
                st_f = (kc == 0)
                sp_f = (kc == 2)
                for qt in range(3):
                    for h in range(4):
                        Ech = E_ts[kc][:, qt, h, :]
                        nc.tensor.matmul(a_ap[:, qt, h, :], Ech,
                                         vf_sb[:, kc, 32 * h:32 * h + 32],
                                         start=st_f, stop=sp_f)
                        nc.tensor.matmul(l_ap[:, qt, h:h + 1], Ech,
                                         ones_sb,
                                         start=st_f, stop=sp_f)

            # ---- normalize: an = a * recip(l) (recip on DVE, mult on Pool)
            rcp = anp.tile([128, 3, 4], F32, tag="rcp")
            nc.vector.reciprocal(rcp[:], l_ap)
            an_sb = anp.tile([128, 3, 4, 32], F16, tag="an")
            for qt in range(3):
                nc.vector.tensor_mul(
                    an_sb[:, qt], a_ap[:, qt],
                    rcp[:, qt].unsqueeze(2).broadcast_to([128, 4, 32]))

            # ---- mean over n (indicator matmul) + output projection
            for qt in range(3):
                nc.tensor.matmul(amT_ap, an_sb[:, qt].rearrange(
                    "p h e -> p (h e)"), sel_sb,
                    start=(qt == 0), stop=(qt == 2))
            amT_sb = anp.tile([128, W], F16, tag="amT")
            nc.gpsimd.tensor_copy(amT_sb[:], amT_ap)
            nc.tensor.matmul(z_ap, wp_sb, amT_sb[:], start=True, stop=True)
            nc.gpsimd.tensor_copy(zo4[:, j, :], z_ap)
          nc.sync.dma_start(out_d[:, gs], zo4[:])
          cell0 += ncell_g

    nc.compile()
    return nc


_NC_CACHE = {}


def _get_nc(n_cells: int):
    if n_cells not in _NC_CACHE:
        _NC_CACHE[n_cells] = _build(n_cells)
    return _NC_CACHE[n_cells]


def _fold_weights(head_gate, lnq_g, lnq_b, lnk_g, lnk_b, lnv_g, lnv_b,
                  Wq, bq, Wk, bk, Wv, bv, Wp, bp):
    """Fold LN affine, head gate, scale, and 1/6-mean into the weights."""
    scale = DH ** -0.5
    gh = np.repeat(np.asarray(head_gate, np.float64), DH)        # [H*DH]
    sq = np.sqrt(scale)

    def proj(g, b, Wx, bx, colscale):
        Wf = (np.asarray(g, np.float64)[:, None] * np.asarray(Wx, np.float64)) * colscale
        bf = (np.asarray(b, np.float64) @ np.asarray(Wx, np.float64)
              + np.asarray(bx, np.float64)) * colscale
        return Wf, bf

    Wq2, bq2 = proj(lnq_g, lnq_b, Wq, bq, gh * sq)
    Wk2, bk2 = proj(lnk_g, lnk_b, Wk, bk, gh * sq)
    Wv2, bv2 = proj(lnv_g, lnv_b, Wv, bv, gh)
    Wp2 = np.asarray(Wp, np.float64) / 6.0
    bp2 = np.asarray(bp, np.float64) + bv2 @ np.asarray(Wp, np.float64)
    return (Wq2.astype(np.float16), bq2.astype(np.float32),
            Wk2.astype(np.float16), bk2.astype(np.float32),
            Wv2.astype(np.float16),
            Wp2.astype(np.float16), bp2.astype(np.float32))


def make_in_maps(q, k, v, skip, head_gate,
                 lnq_g, lnq_b, lnk_g, lnk_b, lnv_g, lnv_b,
                 Wq, bq, Wk, bk, Wv, bv, Wp, bp):
    q = np.asarray(q); k = np.asarray(k); v = np.asarray(v)
    Wq2, bq2, Wk2, bk2, Wv2, Wp2, bp2 = _fold_weights(
        head_gate, lnq_g, lnq_b, lnk_g, lnk_b, lnv_g, lnv_b,
        Wq, bq, Wk, bk, Wv, bv, Wp, bp)

    # device layout: x_in[p=(o,w1,w2), cell=(b,x,y), tensor, c, d], n = 2c+o
    arr = np.stack([q, k, v], axis=1)               # [B, 3, N, X, Y, W1, W2, D]
    arr = arr.reshape(B, 3, 3, 2, X, Y, W1, W2, D)  # N -> (c, o)
    arr = arr.transpose(3, 6, 7, 0, 4, 5, 1, 2, 8)  # [o, w1, w2, B, X, Y, 3, c, d]
    arr = np.ascontiguousarray(
        arr.reshape(128, B * L, 3, 3, D).astype(np.float16))

    sel = np.zeros((128, W), np.float16)
    sel[np.arange(128), np.arange(128) % W] = 1.0
    c16 = np.concatenate(
        [Wq2, Wk2, Wv2, Wp2, sel, np.ones((128, 1), np.float16)],
        axis=1)                                      # [128, 577]
    cst = np.zeros((128, 1168), np.uint8)
    cst[:, 0:1154] = c16.view(np.uint8)
    cst[:, 1156:1160] = bq2.reshape(128, 1).view(np.uint8)
    cst[:, 1160:1164] = bk2.reshape(128, 1).view(np.uint8)
    cst[:, 1164:1168] = np.full((128, 1), RSQRT_MAGIC, np.int32).view(np.uint8)

    consts = {"cst": cst}
    in_maps = []
    for r in range(NCORES):
        s = slice(r * CELLS, (r + 1) * CELLS)
        in_maps.append({
            "x_in": np.ascontiguousarray(arr[:, s]),
            **consts,
        })
    return in_maps, bp2


def kernel(**inputs):
    in_maps, bp2 = make_in_maps(**inputs)
    skip = np.asarray(inputs["skip"])
    nc = _get_nc(CELLS)
    res = run_bass_kernel_spmd(nc, in_maps, core_ids=list(range(NCORES)))
    outs = np.stack([res.results[r]["out"] for r in range(NCORES)])  # [8,D,16,W]
    z = outs.transpose(0, 2, 3, 1).reshape(B * L, W, D)              # [128,W,D]
    z = z + bp2[None, None, :].astype(np.float32)
    z = z.reshape(B, X, Y, W1, W2, D).astype(np.float32) + skip
    return z
